# revision 1
# baseline (speedup 1.0000x reference)
"""Trainium2 Bass kernel for nn_Crude_Diag: y = x @ W.T with W strictly diagonal.

Since W is diagonal, y[i, j] = x[i, j] * diag(W)[j] — a memory-bound
column-wise scale. Strategy (per sharding hint): data-parallel over the token
dim across 8 NeuronCores; the length-n diagonal is replicated to every core.

Each core streams its [1024, 4096] f32 shard through SBUF in [128, 4096]
tiles, multiplies by the diagonal (pre-broadcast across the 128 partitions),
and streams the result back to HBM.
"""

import numpy as np

import concourse.bacc as bacc
import concourse.mybir as mybir
import concourse.tile as tile
from concourse.bass_utils import run_bass_kernel_spmd

TOKENS = 8192
FEATS = 4096
NCORES = 8
ROWS = TOKENS // NCORES  # rows per core
P = 128  # SBUF partitions

# test.py can flip these to capture an NTFF profile of the run.
PROFILE = False
LAST_RESULTS = None

_nc_cache = None


def _build_bass():
    """Build + compile the per-core Bass module (cached across calls)."""
    global _nc_cache
    if _nc_cache is not None:
        return _nc_cache

    nc = bacc.Bacc("TRN2", target_bir_lowering=False, debug=False)
    x = nc.dram_tensor("x", [ROWS, FEATS], mybir.dt.float32, kind="ExternalInput")
    d = nc.dram_tensor("d", [P, FEATS], mybir.dt.float32, kind="ExternalInput")
    y = nc.dram_tensor("y", [ROWS, FEATS], mybir.dt.float32, kind="ExternalOutput")

    with tile.TileContext(nc) as tc:
        with (
            tc.tile_pool(name="const", bufs=1) as cpool,
            tc.tile_pool(name="io", bufs=4) as pool,
        ):
            dscale = cpool.tile([P, FEATS], mybir.dt.float32)
            nc.sync.dma_start(out=dscale[:], in_=d[:])
            for i in range(ROWS // P):
                t = pool.tile([P, FEATS], mybir.dt.float32)
                nc.sync.dma_start(out=t[:], in_=x[i * P:(i + 1) * P, :])
                nc.vector.tensor_mul(out=t[:], in0=t[:], in1=dscale[:])
                nc.scalar.dma_start(out=y[i * P:(i + 1) * P, :], in_=t[:])

    nc.compile()
    _nc_cache = nc
    return nc


def kernel(x: np.ndarray, W: np.ndarray) -> np.ndarray:
    global LAST_RESULTS
    x = np.ascontiguousarray(np.asarray(x, dtype=np.float32))
    W = np.asarray(W, dtype=np.float32)
    assert x.shape == (TOKENS, FEATS), x.shape

    # y = x @ W.T with diagonal W collapses to scaling column j by W[j, j].
    diag = np.ascontiguousarray(np.diagonal(W)).astype(np.float32)
    dscale = np.ascontiguousarray(np.broadcast_to(diag, (P, FEATS)))

    nc = _build_bass()
    in_maps = [
        {"x": x[c * ROWS:(c + 1) * ROWS], "d": dscale} for c in range(NCORES)
    ]
    res = run_bass_kernel_spmd(
        nc, in_maps, core_ids=list(range(NCORES)), trace=PROFILE
    )
    LAST_RESULTS = res
    return np.concatenate([r["y"] for r in res.results], axis=0)


# revision 2
# speedup vs baseline: 1.0612x; 1.0612x over previous
"""Trainium2 Bass kernel for nn_Crude_Diag: y = x @ W.T with W strictly diagonal.

Since W is diagonal, y[i, j] = x[i, j] * diag(W)[j] — a memory-bound
column-wise scale. Strategy (per sharding hint): data-parallel over the token
dim across 8 NeuronCores; the length-n diagonal is replicated to every core.

Per core: the [1024, 4096] f32 shard streams through SBUF in [128, 4096]
tiles. Every load/store is split in half across both HWDGE rings (SP + ACT)
— a single ring tops out around ~215 GB/s, both together sustain the
~435 GB/s SBUF fabric ceiling. The diagonal is shipped as a 16 KiB [1, 4096]
row and broadcast across the 128 partitions on-chip with a ones-matmul on
the (otherwise idle) tensor engine, which is bit-exact for f32.
"""

import numpy as np

import concourse.bacc as bacc
import concourse.mybir as mybir
import concourse.tile as tile
from concourse.bass_utils import run_bass_kernel_spmd

TOKENS = 8192
FEATS = 4096
NCORES = 8
ROWS = TOKENS // NCORES  # rows per core
P = 128  # SBUF partitions
H = FEATS // 2  # half the free dim: one half per HWDGE ring

# test.py can flip these to capture an NTFF profile of the run.
PROFILE = False
LAST_RESULTS = None

_nc_cache = None


def _build_bass():
    """Build + compile the per-core Bass module (cached across calls)."""
    global _nc_cache
    if _nc_cache is not None:
        return _nc_cache

    nc = bacc.Bacc("TRN2", target_bir_lowering=False, debug=False)
    x = nc.dram_tensor("x", [ROWS, FEATS], mybir.dt.float32, kind="ExternalInput")
    d = nc.dram_tensor("d", [1, FEATS], mybir.dt.float32, kind="ExternalInput")
    y = nc.dram_tensor("y", [ROWS, FEATS], mybir.dt.float32, kind="ExternalOutput")

    with tile.TileContext(nc) as tc:
        with (
            tc.tile_pool(name="const", bufs=1) as cpool,
            tc.tile_pool(name="psum", bufs=1, space="PSUM") as ppool,
            tc.tile_pool(name="io", bufs=8) as pool,
        ):
            # Broadcast the diagonal across all 128 partitions via
            # ones[128,1] @ diag[1,512] per PSUM bank (exact for f32).
            diag_row = cpool.tile([1, FEATS], mybir.dt.float32)
            nc.sync.dma_start(out=diag_row[:], in_=d[:])
            ones = cpool.tile([1, P], mybir.dt.float32)
            nc.vector.memset(ones[:], 1.0)
            pd = ppool.tile([P, FEATS], mybir.dt.float32)
            for j in range(FEATS // 512):
                nc.tensor.matmul(
                    pd[:, j * 512:(j + 1) * 512],
                    ones[:],
                    diag_row[:, j * 512:(j + 1) * 512],
                    start=True, stop=True,
                )
            diagB = cpool.tile([P, FEATS], mybir.dt.float32)
            nc.vector.tensor_copy(out=diagB[:], in_=pd[:])

            for i in range(ROWS // P):
                t = pool.tile([P, FEATS], mybir.dt.float32)
                rs = slice(i * P, (i + 1) * P)
                nc.sync.dma_start(out=t[:, 0:H], in_=x[rs, 0:H])
                nc.scalar.dma_start(out=t[:, H:], in_=x[rs, H:])
                nc.vector.tensor_mul(out=t[:], in0=t[:], in1=diagB[:])
                nc.sync.dma_start(out=y[rs, 0:H], in_=t[:, 0:H])
                nc.scalar.dma_start(out=y[rs, H:], in_=t[:, H:])

    nc.compile()
    _nc_cache = nc
    return nc


def kernel(x: np.ndarray, W: np.ndarray) -> np.ndarray:
    global LAST_RESULTS
    x = np.ascontiguousarray(np.asarray(x, dtype=np.float32))
    W = np.asarray(W, dtype=np.float32)
    assert x.shape == (TOKENS, FEATS), x.shape

    # y = x @ W.T with diagonal W collapses to scaling column j by W[j, j].
    diag = np.ascontiguousarray(np.diagonal(W)).astype(np.float32).reshape(1, FEATS)

    nc = _build_bass()
    in_maps = [
        {"x": x[c * ROWS:(c + 1) * ROWS], "d": diag} for c in range(NCORES)
    ]
    res = run_bass_kernel_spmd(
        nc, in_maps, core_ids=list(range(NCORES)), trace=PROFILE
    )
    LAST_RESULTS = res
    return np.concatenate([r["y"] for r in res.results], axis=0)


# revision 5
# speedup vs baseline: 1.0993x; 1.0360x over previous
"""Trainium2 Bass kernel for nn_Crude_Diag: y = x @ W.T with W strictly diagonal.

Since W is diagonal, y[i, j] = x[i, j] * diag(W)[j] — a memory-bound
column-wise scale. Strategy (per sharding hint): data-parallel over the token
dim across 8 NeuronCores; the length-n diagonal is replicated to every core.

Per core: the [1024, 4096] f32 shard streams through SBUF in [128, 4096]
tiles. Every load/store is split in half across both HWDGE rings (SP + ACT)
— a single ring tops out around ~215 GB/s, both together sustain the
~435 GB/s SBUF fabric ceiling. The diagonal is shipped as a 16 KiB [1, 4096]
row and broadcast across the 128 partitions on-chip with a ones-matmul on
the (otherwise idle) tensor engine, which is bit-exact for f32.
"""

import numpy as np

import concourse.bacc as bacc
import concourse.mybir as mybir
import concourse.tile as tile
from concourse.bass_utils import run_bass_kernel_spmd

TOKENS = 8192
FEATS = 4096
NCORES = 8
ROWS = TOKENS // NCORES  # rows per core
P = 128  # SBUF partitions
H = FEATS // 2  # half the free dim: one half per HWDGE ring

# test.py can flip these to capture an NTFF profile of the run.
PROFILE = False
LAST_RESULTS = None

_nc_cache = None


def _build_bass():
    """Build + compile the per-core Bass module (cached across calls)."""
    global _nc_cache
    if _nc_cache is not None:
        return _nc_cache

    nc = bacc.Bacc("TRN2", target_bir_lowering=False, debug=False)
    x = nc.dram_tensor("x", [ROWS, FEATS], mybir.dt.float32, kind="ExternalInput")
    d = nc.dram_tensor("d", [P, FEATS], mybir.dt.float32, kind="ExternalInput")
    y = nc.dram_tensor("y", [ROWS, FEATS], mybir.dt.float32, kind="ExternalOutput")

    NT = ROWS // P
    with tile.TileContext(nc) as tc:
        with (
            tc.tile_pool(name="const", bufs=1) as cpool,
            tc.tile_pool(name="io", bufs=NT) as pool,
        ):
            # Diagonal, pre-broadcast on the host to [128, FEATS]. Loaded via
            # the gpsimd SWDGE queue so it doesn't occupy the two HWDGE rings
            # that stream x/y.
            diagB = cpool.tile([P, FEATS], mybir.dt.float32)
            nc.gpsimd.dma_start(out=diagB[:], in_=d[:])

            # All loads first (one SBUF slot per tile), then the multiplies,
            # then all stores — so late loads are never queued behind stores.
            tiles = []
            for i in range(NT):
                t = pool.tile([P, FEATS], mybir.dt.float32)
                rs = slice(i * P, (i + 1) * P)
                nc.sync.dma_start(out=t[:, 0:H], in_=x[rs, 0:H])
                nc.scalar.dma_start(out=t[:, H:], in_=x[rs, H:])
                tiles.append(t)
            for t in tiles:
                nc.vector.tensor_mul(out=t[:], in0=t[:], in1=diagB[:])
            for i, t in enumerate(tiles):
                rs = slice(i * P, (i + 1) * P)
                nc.sync.dma_start(out=y[rs, 0:H], in_=t[:, 0:H])
                nc.scalar.dma_start(out=y[rs, H:], in_=t[:, H:])

    nc.compile()
    _nc_cache = nc
    return nc


def kernel(x: np.ndarray, W: np.ndarray) -> np.ndarray:
    global LAST_RESULTS
    x = np.ascontiguousarray(np.asarray(x, dtype=np.float32))
    W = np.asarray(W, dtype=np.float32)
    assert x.shape == (TOKENS, FEATS), x.shape

    # y = x @ W.T with diagonal W collapses to scaling column j by W[j, j].
    diag = np.ascontiguousarray(np.diagonal(W)).astype(np.float32)
    dscale = np.ascontiguousarray(np.broadcast_to(diag, (P, FEATS)))

    nc = _build_bass()
    in_maps = [
        {"x": x[c * ROWS:(c + 1) * ROWS], "d": dscale} for c in range(NCORES)
    ]
    res = run_bass_kernel_spmd(
        nc, in_maps, core_ids=list(range(NCORES)), trace=PROFILE
    )
    LAST_RESULTS = res
    return np.concatenate([r["y"] for r in res.results], axis=0)


# revision 6
# speedup vs baseline: 1.1090x; 1.0088x over previous
"""Trainium2 Bass kernel for nn_Crude_Diag: y = x @ W.T with W strictly diagonal.

Since W is diagonal, y[i, j] = x[i, j] * diag(W)[j] — a memory-bound
column-wise scale. Strategy (per sharding hint): data-parallel over the token
dim across 8 NeuronCores; the length-n diagonal is replicated to every core.

Per core: the [1024, 4096] f32 shard streams through SBUF in [128, 4096]
tiles. Every load/store is split in half across both HWDGE rings (SP + ACT)
— a single ring tops out around ~215 GB/s, both together sustain the
~435 GB/s SBUF fabric ceiling. The diagonal is shipped as a 16 KiB [1, 4096]
row and broadcast across the 128 partitions on-chip with a ones-matmul on
the (otherwise idle) tensor engine, which is bit-exact for f32.
"""

import numpy as np

import concourse.bacc as bacc
import concourse.mybir as mybir
import concourse.tile as tile
from concourse.bass_utils import run_bass_kernel_spmd

TOKENS = 8192
FEATS = 4096
NCORES = 8
ROWS = TOKENS // NCORES  # rows per core
P = 128  # SBUF partitions
H = FEATS // 2  # half the free dim: one half per HWDGE ring

# test.py can flip these to capture an NTFF profile of the run.
PROFILE = False
LAST_RESULTS = None

_nc_cache = None


def _build_bass():
    """Build + compile the per-core Bass module (cached across calls)."""
    global _nc_cache
    if _nc_cache is not None:
        return _nc_cache

    nc = bacc.Bacc("TRN2", target_bir_lowering=False, debug=False)
    x = nc.dram_tensor("x", [ROWS, FEATS], mybir.dt.float32, kind="ExternalInput")
    d = nc.dram_tensor("d", [P, FEATS], mybir.dt.float32, kind="ExternalInput")
    y = nc.dram_tensor("y", [ROWS, FEATS], mybir.dt.float32, kind="ExternalOutput")

    NT = ROWS // P
    # One whole [128, 4096] tile (2 MiB) per dma_start, spread over the three
    # available DMA queues: sync (HWDGE q1), scalar (HWDGE q10), gpsimd
    # (SWDGE q0). Any single queue tops out near ~215 GB/s; three together
    # saturate the ~435 GB/s SBUF fabric.
    load_eng = ["gpsimd", "sync", "scalar", "gpsimd", "sync", "scalar", "sync", "scalar"]
    store_eng = ["scalar", "sync", "gpsimd", "scalar", "sync", "gpsimd", "scalar", "gpsimd"]
    with tile.TileContext(nc) as tc:
        with (
            tc.tile_pool(name="const", bufs=1) as cpool,
            tc.tile_pool(name="io", bufs=NT) as pool,
        ):
            # Diagonal, pre-broadcast on the host to [128, FEATS].
            diagB = cpool.tile([P, FEATS], mybir.dt.float32)
            nc.gpsimd.dma_start(out=diagB[:], in_=d[:])

            # All loads first (one SBUF slot per tile), then the multiplies,
            # then all stores — so late loads are never queued behind stores.
            tiles = []
            for i in range(NT):
                t = pool.tile([P, FEATS], mybir.dt.float32)
                rs = slice(i * P, (i + 1) * P)
                getattr(nc, load_eng[i]).dma_start(out=t[:], in_=x[rs, :])
                tiles.append(t)
            for t in tiles:
                nc.vector.tensor_mul(out=t[:], in0=t[:], in1=diagB[:])
            for i, t in enumerate(tiles):
                rs = slice(i * P, (i + 1) * P)
                getattr(nc, store_eng[i]).dma_start(out=y[rs, :], in_=t[:])

    nc.compile()
    _nc_cache = nc
    return nc


def kernel(x: np.ndarray, W: np.ndarray) -> np.ndarray:
    global LAST_RESULTS
    x = np.ascontiguousarray(np.asarray(x, dtype=np.float32))
    W = np.asarray(W, dtype=np.float32)
    assert x.shape == (TOKENS, FEATS), x.shape

    # y = x @ W.T with diagonal W collapses to scaling column j by W[j, j].
    diag = np.ascontiguousarray(np.diagonal(W)).astype(np.float32)
    dscale = np.ascontiguousarray(np.broadcast_to(diag, (P, FEATS)))

    nc = _build_bass()
    in_maps = [
        {"x": x[c * ROWS:(c + 1) * ROWS], "d": dscale} for c in range(NCORES)
    ]
    res = run_bass_kernel_spmd(
        nc, in_maps, core_ids=list(range(NCORES)), trace=PROFILE
    )
    LAST_RESULTS = res
    return np.concatenate([r["y"] for r in res.results], axis=0)
